# revision 6
# baseline (speedup 1.0000x reference)
"""Trainium2 Bass kernel for nn_EnergyLinearCQN.

Data-parallel over the batch across 8 NeuronCores. Device d owns batch rows
{t*8192 + d*1024 + c*128 + p : t,c in [0,8), p in [0,128)} — i.e. a stripe of
1024 "u"-positions replicated across the 8 preference groups, so the envelope
combine (which couples rows u, u+8192, ..., u+7*8192) is core-local.

Local row order on each device: l = c*1024 + t*128 + p  (c = u-chunk of 128,
t = preference-group index, p = partition). This makes the combine for u-chunk
c depend only on MLP row-chunks 2c and 2c+1, so the softmax combine pipelines
with the MLP of later chunks.

Per-core pipeline:
  xT [32, 8192] (host-transposed, zero-padded to 128 partitions on SBUF)
  L1: h1^T = relu(W1p.T @ xT + b1)            fp32r matmuls, K=128(padded)
  L2: h2^T = relu(W2.T @ h1^T + b2)           fp32r, K=512
  L3: q = h2 @ W3 + b3 (row-major tiles)      bf16, K=1024, N=64
  combine: per (u-chunk c, group t):
      s   = C0 + (p1/p0)*C1                   one scalar_tensor_tensor
      e   = exp(10*p0*s), Z = sum(e)          one ACT op w/ accumulator
      h_r = sum(e*C_r)                        two stt ops w/ accumulator
      hq  = h_r / Z
  where C_r[u, (t', a)] are strided views of the q SBUF tiles — no gather or
  transpose is materialized.
"""

import numpy as np
import ml_dtypes

import concourse.bass as bass
import concourse.mybir as mybir
import concourse.tile as tile_mod
from concourse.tile import TileContext
from concourse.vector_clock import ScopedClock
from concourse import bass2jax
from concourse.bass2jax import _bass_exec_p, partition_id_tensor

F32 = mybir.dt.float32
F32R = mybir.dt.float32r
BF16 = mybir.dt.bfloat16
AF = mybir.ActivationFunctionType
OP = mybir.AluOpType

NCORES = 8
W_NUM = 8
S_NUM = 8192
SLOC = 1024            # u positions per core
ROWS = 8192            # local rows per core
A = 32
R = 2
D = 32
H1 = 512
H2 = 1024
NCH = 16               # MLP row chunks of 512
ALPHA_INV = 10.0

# ---------------------------------------------------------------------------
# Workarounds for this walrus build: at most 1 sync-wait per instruction
# (2 for EventSemaphore). Split excess waits onto preceding same-engine nops.
# ---------------------------------------------------------------------------


def _fix_multiwait(nc):
    for f in nc.m.functions:
        for bb in f.blocks:
            insts = bb.instructions
            i = 0
            while i < len(insts):
                ins = insts[i]
                si = ins.sync_info
                waits = list(si.on_wait or []) if si is not None else []
                cap = 2 if type(ins).__name__ == "InstEventSemaphore" else 1
                if len(waits) > cap:
                    keep = waits[-cap:]
                    extra = waits[:-cap]
                    ins.sync_info.on_wait = keep
                    for j, w in enumerate(extra):
                        nop = mybir.InstNoOp(name=nc.get_next_instruction_name())
                        nop.engine = ins.engine
                        nop.sync_info = mybir.SyncInfo(on_wait=[w], on_update=[])
                        insts.insert(i + j, nop)
                    i += len(extra)
                i += 1


def _patched_drain_and_barrier(self, tick_clock, wait_clock):
    nc = self.nc
    drain_inst = nc.sync.drain()
    wait_clock.add_sem_waits(
        drain_inst.ins, ScopedClock({None: tick_clock.global_clock})
    )
    si = drain_inst.ins.sync_info
    waits = list(si.on_wait or []) if si is not None else []
    if len(waits) > 1:
        drain_inst.ins.sync_info.on_wait = [waits[-1]]
        insts = nc.cur_bb.bb.instructions
        drain_pos = len(insts) - 1
        assert insts[drain_pos] is drain_inst.ins
        extra = []
        for w in waits[:-1]:
            nop = nc.sync.nop(nofuse=True)
            nop.ins.sync_info = mybir.SyncInfo(on_wait=[w], on_update=[])
            extra.append(nop.ins)
        for _ in extra:
            insts.pop()
        for i, e in enumerate(extra):
            insts.insert(drain_pos + i, e)
    nc.all_engine_barrier()
    assert self.sems is not None
    popped = nc._tile_sem_poison_stack.pop()
    assert popped is self._sem_poison
    nc.clear_and_free_semaphores(list(self.sems.allocated().values()))
    nc.all_engine_barrier()


tile_mod.TileContext._drain_and_barrier = _patched_drain_and_barrier


# ---------------------------------------------------------------------------
# Device program (identical on all 8 cores; data differs per core)
# ---------------------------------------------------------------------------


def build_nc(loop_K=None, fix_waits=True):
    nc = bass.Bass()

    xT_in = nc.dram_tensor("xT_in", [D, ROWS], F32R, kind="ExternalInput")
    p01_in = nc.dram_tensor("p01_in", [128, 128], F32, kind="ExternalInput")
    w1_in = nc.dram_tensor("w1_in", [128, H1], F32R, kind="ExternalInput")
    w2_in = nc.dram_tensor("w2_in", [H1, H2], F32R, kind="ExternalInput")
    w3_in = nc.dram_tensor("w3_in", [H2, A * R], BF16, kind="ExternalInput")
    b1_in = nc.dram_tensor("b1_in", [128, 4], F32, kind="ExternalInput")
    b2_in = nc.dram_tensor("b2_in", [128, 8], F32, kind="ExternalInput")
    b3_in = nc.dram_tensor("b3_in", [128, A * R], F32, kind="ExternalInput")
    q_out = nc.dram_tensor("q_out", [ROWS, A * R], F32, kind="ExternalOutput")
    hq_out = nc.dram_tensor("hq_out", [ROWS, R], F32, kind="ExternalOutput")

    with TileContext(nc) as tc:
        with (
            tc.tile_pool(name="wp", bufs=1) as wp,
            tc.tile_pool(name="qsb", bufs=1) as qsbp,
            tc.tile_pool(name="h1s", bufs=2) as h1sp,
            tc.tile_pool(name="h2s", bufs=2) as h2sp,
            tc.tile_pool(name="ps1", bufs=2, space="PSUM") as ps1,
            tc.tile_pool(name="ps2", bufs=4, space="PSUM") as ps2,
            tc.tile_pool(name="psq", bufs=2, space="PSUM") as psq,
            tc.tile_pool(name="comb", bufs=3) as comb,
            tc.tile_pool(name="small", bufs=10) as small,
        ):
            # ---- persistent tensors -------------------------------------
            xsb = wp.tile([128, ROWS], F32R, tag="xsb")
            for qd in range(3):
                nc.vector.memset(
                    xsb[D + 32 * qd:D + 32 * (qd + 1), :].bitcast(F32), 0.0
                )
            for part in range(4):
                sl = slice(part * (ROWS // 4), (part + 1) * (ROWS // 4))
                nc.sync.dma_start(out=xsb[0:D, sl], in_=xT_in[:, sl])

            w1sb = wp.tile([128, H1], F32R, tag="w1sb")
            nc.sync.dma_start(out=w1sb, in_=w1_in[:, :])
            w2sb = wp.tile([128, 4, H2], F32R, tag="w2sb")
            for kc in range(4):
                nc.sync.dma_start(
                    out=w2sb[:, kc, :], in_=w2_in[kc * 128:(kc + 1) * 128, :]
                )
            w3sb = wp.tile([128, 8, A * R], BF16, tag="w3sb")
            for kc in range(8):
                nc.sync.dma_start(
                    out=w3sb[:, kc, :], in_=w3_in[kc * 128:(kc + 1) * 128, :]
                )
            b1sb = wp.tile([128, 4], F32, tag="b1sb")
            nc.sync.dma_start(out=b1sb, in_=b1_in[:, :])
            b2sb = wp.tile([128, 8], F32, tag="b2sb")
            nc.sync.dma_start(out=b2sb, in_=b2_in[:, :])
            b3sb = wp.tile([128, A * R], F32, tag="b3sb")
            nc.sync.dma_start(out=b3sb, in_=b3_in[:, :])
            p01sb = wp.tile([128, 8, 8, 2], F32, tag="p01sb")
            nc.sync.dma_start(
                out=p01sb, in_=p01_in.rearrange("p (c t r) -> p c t r", t=8, r=2)
            )

            q_sb = qsbp.tile([128, 64, A * R], F32, tag="q_sb")
            qv = q_sb.rearrange("p t (a r) -> p t a r", r=2)

            # ---- MLP chunk (512 rows) -----------------------------------
            def mlp_chunk(n):
                xt = xsb[:, n * 512:(n + 1) * 512]
                h1sb = h1sp.tile([128, 4, 512], F32R, tag="h1sb")
                for m in range(4):
                    ps = ps1.tile([128, 512], F32, tag="ps1t")
                    nc.tensor.matmul(
                        ps, w1sb[:, bass.ts(m, 128)], xt, start=True, stop=True
                    )
                    nc.scalar.activation(
                        out=h1sb[:, m, :], in_=ps, func=AF.Relu,
                        bias=b1sb[:, m:m + 1], scale=1.0,
                    )
                h2sb = h2sp.tile([128, 8, 512], BF16, tag="h2sb")
                for m in range(8):
                    ps = ps2.tile([128, 512], F32, tag="ps2t")
                    for kc in range(4):
                        nc.tensor.matmul(
                            ps,
                            w2sb[:, kc, bass.ts(m, 128)],
                            h1sb[:, kc, :],
                            start=(kc == 0),
                            stop=(kc == 3),
                        )
                    if m % 4 == 3:
                        # spread some relu copies to DVE to balance engines
                        nc.vector.tensor_scalar(
                            out=h2sb[:, m, :], in0=ps,
                            scalar1=b2sb[:, m:m + 1], scalar2=0.0,
                            op0=OP.add, op1=OP.max,
                        )
                    else:
                        nc.scalar.activation(
                            out=h2sb[:, m, :], in_=ps, func=AF.Relu,
                            bias=b2sb[:, m:m + 1], scale=1.0,
                        )
                for rt in range(4):
                    tidx = n * 4 + rt
                    ps = psq.tile([128, A * R], F32, tag="psqt")
                    for kc in range(8):
                        nc.tensor.matmul(
                            ps,
                            h2sb[:, kc, bass.ts(rt, 128)],
                            w3sb[:, kc, :],
                            start=(kc == 0),
                            stop=(kc == 7),
                        )
                    nc.vector.scalar_tensor_tensor(
                        out=q_sb[:, tidx, :], in0=ps, scalar=1.0, in1=b3sb,
                        op0=OP.mult, op1=OP.add,
                    )
                    nc.sync.dma_start(
                        out=q_out[tidx * 128:(tidx + 1) * 128, :],
                        in_=q_sb[:, tidx, :],
                    )

            # ---- envelope combine for u-chunk c -------------------------
            def combine(c):
                p0 = p01sb[:, c, :, 0]
                p1 = p01sb[:, c, :, 1]
                p0m = small.tile([128, 8], F32, tag="p0m")
                nc.vector.tensor_scalar_max(p0m, p0, 1e-9)
                rec = small.tile([128, 8], F32, tag="rec")
                nc.vector.reciprocal(rec, p0m)
                ratio = small.tile([128, 8], F32, tag="ratio")
                nc.vector.tensor_mul(ratio, rec, p1)
                p10 = small.tile([128, 8], F32, tag="p10")
                nc.vector.tensor_scalar_mul(p10, p0m, ALPHA_INV)
                zsum = small.tile([128, 8], F32, tag="zsum")
                h0 = small.tile([128, 8], F32, tag="h0")
                h1c = small.tile([128, 8], F32, tag="h1c")
                c0 = qv[:, c * 8:(c + 1) * 8, :, 0]
                c1 = qv[:, c * 8:(c + 1) * 8, :, 1]
                for t in range(8):
                    s = comb.tile([128, 8, 32], F32, tag="s")
                    nc.vector.scalar_tensor_tensor(
                        out=s, in0=c1, scalar=ratio[:, t:t + 1], in1=c0,
                        op0=OP.mult, op1=OP.add,
                    )
                    e = comb.tile([128, 8, 32], F32, tag="e")
                    nc.scalar.activation(
                        out=e, in_=s, func=AF.Exp,
                        bias=0.0, scale=p10[:, t:t + 1],
                        accum_out=zsum[:, t:t + 1],
                    )
                    scr = comb.tile([128, 8, 32], F32, tag="scr")
                    nc.vector.scalar_tensor_tensor(
                        out=scr, in0=e, scalar=1.0, in1=c0,
                        op0=OP.mult, op1=OP.mult,
                        accum_out=h0[:, t:t + 1],
                    )
                    scr2 = comb.tile([128, 8, 32], F32, tag="scr2")
                    nc.vector.scalar_tensor_tensor(
                        out=scr2, in0=e, scalar=1.0, in1=c1,
                        op0=OP.mult, op1=OP.mult,
                        accum_out=h1c[:, t:t + 1],
                    )
                zr = small.tile([128, 8], F32, tag="zr")
                nc.vector.reciprocal(zr, zsum)
                hqc = small.tile([128, 8, 2], F32, tag="hqc")
                nc.vector.tensor_mul(hqc[:, :, 0], h0, zr)
                nc.vector.tensor_mul(hqc[:, :, 1], h1c, zr)
                nc.sync.dma_start(
                    out=hq_out[c * SLOC:(c + 1) * SLOC, :].rearrange(
                        "(t p) r -> p t r", p=128
                    ),
                    in_=hqc,
                )

            def whole_body():
                for c in range(8):
                    mlp_chunk(2 * c)
                    mlp_chunk(2 * c + 1)
                    combine(c)

            if loop_K is None:
                whole_body()
            else:
                with tc.For_i(0, loop_K, 1):
                    whole_body()

    if fix_waits:
        _fix_multiwait(nc)
    return nc


# ---------------------------------------------------------------------------
# Host wrapper: shard, run on 8 cores via PJRT, reassemble
# ---------------------------------------------------------------------------

_RUNNER = None


class _Runner:
    def __init__(self):
        import jax
        from jax.sharding import Mesh, PartitionSpec
        from jax.experimental.shard_map import shard_map

        bass2jax.install_neuronx_cc_hook()
        nc = build_nc()
        self.nc = nc
        partition_name = (
            nc.partition_id_tensor.name if nc.partition_id_tensor else None
        )
        in_names, out_names, out_avals, zero_shapes = [], [], [], []
        for alloc in nc.m.functions[0].allocations:
            if not isinstance(alloc, mybir.MemoryLocationSet):
                continue
            name = alloc.memorylocations[0].name
            if alloc.kind == "ExternalInput":
                if name != partition_name:
                    in_names.append(name)
            elif alloc.kind == "ExternalOutput":
                shape = tuple(alloc.tensor_shape)
                dtype = mybir.dt.np(alloc.dtype)
                out_names.append(name)
                out_avals.append(jax.core.ShapedArray(shape, dtype))
                zero_shapes.append((shape, dtype))
        self.in_names, self.out_names = in_names, out_names
        self.out_avals, self.zero_shapes = out_avals, zero_shapes
        all_names = in_names + out_names
        if partition_name is not None:
            all_names = all_names + [partition_name]

        def _body(*args):
            operands = list(args)
            if partition_name is not None:
                operands.append(partition_id_tensor())
            outs = _bass_exec_p.bind(
                *operands,
                out_avals=tuple(out_avals),
                in_names=tuple(all_names),
                out_names=tuple(out_names),
                lowering_input_output_aliases=(),
                sim_require_finite=True,
                sim_require_nnan=True,
                nc=nc,
            )
            return tuple(outs)

        devices = jax.devices()[:NCORES]
        mesh = Mesh(np.asarray(devices), ("core",))
        nin = len(in_names) + len(out_names)
        self._fn = jax.jit(
            shard_map(
                _body,
                mesh=mesh,
                in_specs=(PartitionSpec("core"),) * nin,
                out_specs=(PartitionSpec("core"),) * len(out_names),
                check_rep=False,
            ),
            keep_unused=True,
        )
        self._jax = jax

    def run(self, in_maps):
        jax = self._jax
        concat_in = [
            np.concatenate([np.asarray(m[name]) for m in in_maps], axis=0)
            for name in self.in_names
        ]
        concat_zero = [
            np.zeros((NCORES * s[0], *s[1:]), dt) for (s, dt) in self.zero_shapes
        ]
        outs = self._fn(*concat_in, *concat_zero)
        jax.block_until_ready(outs)
        return [
            {
                name: np.asarray(outs[i]).reshape(
                    NCORES, *self.out_avals[i].shape
                )[c]
                for i, name in enumerate(self.out_names)
            }
            for c in range(NCORES)
        ]


def get_runner():
    global _RUNNER
    if _RUNNER is None:
        _RUNNER = _Runner()
    return _RUNNER


def make_in_maps(state, preference, W1, b1, W2, b2, W3, b3):
    state = np.asarray(state, dtype=np.float32)
    preference = np.asarray(preference, dtype=np.float32)
    W1 = np.asarray(W1, dtype=np.float32)
    b1 = np.asarray(b1, dtype=np.float32)
    W2 = np.asarray(W2, dtype=np.float32)
    b2 = np.asarray(b2, dtype=np.float32)
    W3 = np.asarray(W3, dtype=np.float32)
    b3 = np.asarray(b3, dtype=np.float32)

    w1p = np.zeros((128, H1), np.float32)
    w1p[0:30] = W1[0:30]
    w1p[30:32] = W1[30:32]
    b1r = np.ascontiguousarray(b1.reshape(4, 128).T)
    b2r = np.ascontiguousarray(b2.reshape(8, 128).T)
    b3r = np.ascontiguousarray(np.broadcast_to(b3, (128, A * R)))
    w3bf = W3.astype(ml_dtypes.bfloat16)

    # global row for (d, c, t, p): t*8192 + d*1024 + c*128 + p
    st = state.reshape(W_NUM, NCORES, 8, 128, 30)      # [t, d, c, p, :]
    pr = preference.reshape(W_NUM, NCORES, 8, 128, 2)
    in_maps = []
    for d in range(NCORES):
        xs = np.transpose(st[:, d], (1, 0, 2, 3)).reshape(ROWS, 30)  # [c,t,p]
        ps = np.transpose(pr[:, d], (1, 0, 2, 3)).reshape(ROWS, 2)
        xT = np.empty((D, ROWS), np.float32)
        xT[0:30] = xs.T
        xT[30:32] = ps.T
        # p01_in [p, c*16 + t*2 + r]
        p01 = np.transpose(pr[:, d], (2, 1, 0, 3)).reshape(128, 128)
        p01 = np.ascontiguousarray(p01)
        in_maps.append(
            {
                "xT_in": np.ascontiguousarray(xT),
                "p01_in": p01,
                "w1_in": w1p,
                "w2_in": W2,
                "w3_in": w3bf,
                "b1_in": b1r,
                "b2_in": b2r,
                "b3_in": b3r,
            }
        )
    return in_maps


def assemble(results):
    hq = np.empty((W_NUM, NCORES, 8, 128, R), np.float32)
    q3 = np.empty((W_NUM, NCORES, 8, 128, A, R), np.float32)
    for d in range(NCORES):
        hq_d = results[d]["hq_out"].reshape(8, 8, 128, R)       # [c, t, p, r]
        q_d = results[d]["q_out"].reshape(8, 8, 128, A, R)      # [c, t, p, a, r]
        hq[:, d] = np.transpose(hq_d, (1, 0, 2, 3))
        q3[:, d] = np.transpose(q_d, (1, 0, 2, 3, 4))
    return (
        hq.reshape(W_NUM * NCORES * SLOC, R),
        q3.reshape(W_NUM * NCORES * SLOC, A, R),
    )


def kernel(state, preference, W1, b1, W2, b2, W3, b3, w_num):
    assert int(w_num) == W_NUM
    runner = get_runner()
    in_maps = make_in_maps(state, preference, W1, b1, W2, b2, W3, b3)
    results = runner.run(in_maps)
    return assemble(results)
